# revision 5
# baseline (speedup 1.0000x reference)
"""Trainium2 Bass kernel for the 6-layer transformer encoder.

Sharding: 8 cores = (batch 4) x (seq-half 2). Core c handles batch c//2,
sequence half c%2 (1024 tokens). Per layer, K/V halves are exchanged with
the pair core via a 2-core AllGather through DRAM.

Layout: residual stream x is token-major fp32 (exact). Matmul operands are
fp32r (full PE rate). Per layer:
  prep: x1T = PE-transpose(x)            [feature-major, fp32r]
  QKV:  K^T, Q^T feature-major (Form F: lhsT=W chunk, rhs=x1T chunk)
        V    token-major       (Form T: lhsT=x1T chunk, rhs=Wv chunk)
        V stored with a ones column per head (softmax denominator trick)
  attn: per (head-pair hp, head): E^T[keys,queries] psum [128,1024]
        = mm(lhsT=K^T[64,kt*128:...], rhs=Q^T[64,:]) row-packed via
        tile_position (0,0)/(64,0); P = exp(E/8 + maskbias) (ACT, fused);
        O^T[65,1024] += mm(lhsT=V[:, kt, h, 0:65], rhs=P) over 16 kt.
        Row 64 of O^T = sum over keys of P  ->  r = 1/sum; broadcast r
        across 64 partitions via K=1 ones-matmul; oT = O^T[0:64] * r_bc.
  Wo:   Form T (lhsT=oT chunk, rhs=Wo chunk) -> + bo + residual -> LN1
  FFN:  hT = relu(W1^T@x1T + b1) feature-major (Form F), ff Form T
        (lhsT=hT chunk, rhs=W2 chunk) -> + b2 + residual -> LN2
"""
import sys
import os

sys.path.insert(0, "/opt/trn_rl_repo")

import numpy as np

V, D, L, H, FF = 32000, 512, 6, 8, 2048
HD = D // H            # 64
B, S = 4, 2048
NCORES = 8
TLOC = S // 2          # 1024 tokens per core
NT = TLOC // 128       # 8 token tiles per core
NKT = S // 128         # 16 key tiles (full sequence)
FC = D // 128          # 4 feature chunks
HC = FF // 128         # 16 hidden chunks
EPS = 1e-5
SCALE = 1.0 / np.sqrt(HD)

_PROGRAM = None


def _build_program():
    import concourse.bass as bass
    import concourse.bacc as bacc
    import concourse.mybir as mybir
    from concourse import tile
    from concourse.masks import make_identity

    F32 = mybir.dt.float32
    F32R = mybir.dt.float32r
    I32 = mybir.dt.int32
    AF = mybir.ActivationFunctionType
    OP = mybir.AluOpType

    nc = bacc.Bacc("TRN2", target_bir_lowering=False, debug=False,
                   num_devices=NCORES)

    # ---------------- DRAM parameters ----------------
    emb_ext = nc.declare_dram_parameter("tok_emb", [V, D], F32, isOutput=False)
    idx_ext = nc.declare_dram_parameter("src_idx", [NT, 128, 1], I32, isOutput=False)
    pos_ext = nc.declare_dram_parameter("posenc", [NT, 128, D], F32, isOutput=False)
    mb_ext = nc.declare_dram_parameter("maskb", [NKT, 128, 1], F32, isOutput=False)
    ones_ext = nc.declare_dram_parameter("ones64", [1, 64], F32, isOutput=False)

    wq_ext = nc.declare_dram_parameter("Wq", [L, D, D], F32, isOutput=False)
    bq_ext = nc.declare_dram_parameter("bq", [L, D], F32, isOutput=False)
    wk_ext = nc.declare_dram_parameter("Wk", [L, D, D], F32, isOutput=False)
    bk_ext = nc.declare_dram_parameter("bk", [L, D], F32, isOutput=False)
    wv_ext = nc.declare_dram_parameter("Wv", [L, D, D], F32, isOutput=False)
    bv_ext = nc.declare_dram_parameter("bv", [L, D], F32, isOutput=False)
    wo_ext = nc.declare_dram_parameter("Wo", [L, D, D], F32, isOutput=False)
    bo_ext = nc.declare_dram_parameter("bo", [L, D], F32, isOutput=False)
    w1_ext = nc.declare_dram_parameter("W1", [L, D, FF], F32, isOutput=False)
    b1_ext = nc.declare_dram_parameter("b1", [L, FF], F32, isOutput=False)
    w2_ext = nc.declare_dram_parameter("W2", [L, FF, D], F32, isOutput=False)
    b2_ext = nc.declare_dram_parameter("b2", [L, D], F32, isOutput=False)
    g1_ext = nc.declare_dram_parameter("g1", [L, D], F32, isOutput=False)
    be1_ext = nc.declare_dram_parameter("be1", [L, D], F32, isOutput=False)
    g2_ext = nc.declare_dram_parameter("g2", [L, D], F32, isOutput=False)
    be2_ext = nc.declare_dram_parameter("be2", [L, D], F32, isOutput=False)

    out_ext = nc.declare_dram_parameter("out", [NT, 128, D], F32, isOutput=True)

    # collective buffers (per layer, to avoid cross-layer hazards)
    cc_kin = [nc.dram_tensor(f"cc_kin{l}", [D, TLOC], F32R) for l in range(L)]
    cc_kout = [nc.dram_tensor(f"cc_kout{l}", [2, D, TLOC], F32R) for l in range(L)]
    cc_vin = [nc.dram_tensor(f"cc_vin{l}", [TLOC, H * (HD + 1)], F32R) for l in range(L)]
    cc_vout = [nc.dram_tensor(f"cc_vout{l}", [2, TLOC, H * (HD + 1)], F32R) for l in range(L)]
    groups = [[0, 1], [2, 3], [4, 5], [6, 7]]

    with tile.TileContext(nc, num_cores=NCORES) as tc:
        with (
            tc.tile_pool(name="glob", bufs=1) as glob,
            tc.tile_pool(name="ps", bufs=1, space="PSUM") as pspool,
        ):
            # persistent across layers
            x_t = glob.tile([128, NT, D], F32, tag="x")         # residual stream
            ident = glob.tile([128, 128], F32, tag="ident")
            make_identity(nc, ident[:])
            ones_r = glob.tile([1, 64], F32R, tag="ones64")
            nc.sync.dma_start(out=ones_r[:], in_=ones_ext[:].bitcast(F32R))
            eps_t = glob.tile([128, 1], F32, tag="eps")
            nc.vector.memset(eps_t[:], EPS)
            onecol = glob.tile([128, 1], F32, tag="onecol")
            nc.vector.memset(onecol[:], 1.0)
            mb_t = glob.tile([128, NKT, 1], F32, tag="maskb")
            nc.sync.dma_start(out=mb_t[:], in_=mb_ext.rearrange("k p o -> p k o"))

            def psum(shape, tag):
                return pspool.tile(shape, F32, tag=tag, name=tag)

            # ---------------- embedding ----------------
            with tc.tile_pool(name="emb", bufs=3) as embp:
                for t in range(NT):
                    idx = embp.tile([128, 1], I32, tag="idx")
                    nc.sync.dma_start(out=idx[:], in_=idx_ext[t])
                    g = embp.tile([128, D], F32, tag="gat")
                    nc.gpsimd.indirect_dma_start(
                        out=g[:], out_offset=None, in_=emb_ext[:],
                        in_offset=bass.IndirectOffsetOnAxis(ap=idx[:, :1], axis=0),
                    )
                    p = embp.tile([128, D], F32, tag="pos")
                    nc.sync.dma_start(out=p[:], in_=pos_ext[t])
                    nc.vector.tensor_add(out=x_t[:, t, :], in0=g[:], in1=p[:])

            # ---------------- layers ----------------
            for l in range(L):
                half = l  # just for names
                with tc.tile_pool(name=f"L{l}", bufs=1) as lp:
                    # layer-wide tiles
                    oT = lp.tile([128, FC, TLOC], F32R, tag="oT")
                    wo_sb = lp.tile([128, FC, D], F32R, tag="wo")
                    nc.sync.dma_start(
                        out=wo_sb[:],
                        in_=wo_ext[l].rearrange("(c p) m -> p c m", p=128).bitcast(F32R))
                    bo_bc = lp.tile([128, D], F32, tag="bo_bc")
                    nc.sync.dma_start(
                        out=bo_bc[:],
                        in_=bo_ext[l][None, :].partition_broadcast(128).opt())
                    bv_bc = lp.tile([128, D], F32, tag="bv_bc")
                    nc.sync.dma_start(
                        out=bv_bc[:],
                        in_=bv_ext[l][None, :].partition_broadcast(128).opt())
                    b2_bc = lp.tile([128, D], F32, tag="b2_bc")
                    nc.sync.dma_start(
                        out=b2_bc[:],
                        in_=b2_ext[l][None, :].partition_broadcast(128).opt())
                    g1_bc = lp.tile([128, D], F32, tag="g1_bc")
                    nc.sync.dma_start(
                        out=g1_bc[:],
                        in_=g1_ext[l][None, :].partition_broadcast(128).opt())
                    be1_bc = lp.tile([128, D], F32, tag="be1_bc")
                    nc.sync.dma_start(
                        out=be1_bc[:],
                        in_=be1_ext[l][None, :].partition_broadcast(128).opt())
                    g2_bc = lp.tile([128, D], F32, tag="g2_bc")
                    nc.sync.dma_start(
                        out=g2_bc[:],
                        in_=g2_ext[l][None, :].partition_broadcast(128).opt())
                    be2_bc = lp.tile([128, D], F32, tag="be2_bc")
                    nc.sync.dma_start(
                        out=be2_bc[:],
                        in_=be2_ext[l][None, :].partition_broadcast(128).opt())
                    bq_sb = lp.tile([128, FC], F32, tag="bq")
                    nc.sync.dma_start(out=bq_sb[:],
                                      in_=bq_ext[l].rearrange("(c p) -> p c", p=128))
                    bk_sb = lp.tile([128, FC], F32, tag="bk")
                    nc.sync.dma_start(out=bk_sb[:],
                                      in_=bk_ext[l].rearrange("(c p) -> p c", p=128))
                    b1_sb = lp.tile([128, HC], F32, tag="b1")
                    nc.sync.dma_start(out=b1_sb[:],
                                      in_=b1_ext[l].rearrange("(c p) -> p c", p=128))

                    with tc.tile_pool(name=f"kv{l}", bufs=1) as kvp:
                        q_sb = kvp.tile([128, FC, TLOC], F32R, tag="q")
                        k_sb = kvp.tile([128, FC, S], F32R, tag="k")
                        v_sb = kvp.tile([128, NKT, H, HD + 1], F32R, tag="v")

                        with tc.tile_pool(name=f"qkv{l}", bufs=1) as qkvp:
                            xT = qkvp.tile([128, FC, TLOC], F32R, tag="xT")
                            wq_sb = qkvp.tile([128, FC, D], F32R, tag="wq")
                            nc.sync.dma_start(
                                out=wq_sb[:],
                                in_=wq_ext[l].rearrange("(c p) m -> p c m", p=128).bitcast(F32R))
                            wk_sb = qkvp.tile([128, FC, D], F32R, tag="wk")
                            nc.sync.dma_start(
                                out=wk_sb[:],
                                in_=wk_ext[l].rearrange("(c p) m -> p c m", p=128).bitcast(F32R))
                            wv_sb = qkvp.tile([128, FC, D], F32R, tag="wv")
                            nc.sync.dma_start(
                                out=wv_sb[:],
                                in_=wv_ext[l].rearrange("(c p) m -> p c m", p=128).bitcast(F32R))

                            # prep: transpose x -> xT (fp32r)
                            for t in range(NT):
                                for fc in range(FC):
                                    tp = psum([128, 128], "E")
                                    nc.tensor.transpose(
                                        out=tp[:], in_=x_t[:, t, fc * 128:(fc + 1) * 128],
                                        identity=ident[:])
                                    nc.vector.tensor_copy(
                                        out=xT[:, fc, t * 128:(t + 1) * 128], in_=tp[:])

                            # K^T (first, so the exchange starts early)
                            for m in range(FC):
                                for qc in range(2):
                                    ps = psum([128, 512], "E")
                                    for fc in range(FC):
                                        nc.tensor.matmul(
                                            out=ps[:],
                                            lhsT=wk_sb[:, fc, m * 128:(m + 1) * 128],
                                            rhs=xT[:, fc, qc * 512:(qc + 1) * 512],
                                            start=(fc == 0), stop=(fc == FC - 1))
                                    nc.vector.tensor_scalar(
                                        out=k_sb[:, m, qc * 512:(qc + 1) * 512],
                                        in0=ps[:], scalar1=bk_sb[:, m:m + 1],
                                        scalar2=None, op0=OP.add)
                            # ship local K half, gather pair
                            nc.sync.dma_start(
                                out=cc_kin[l].rearrange("(c p) t -> p c t", p=128),
                                in_=k_sb[:, :, 0:TLOC])
                            nc.gpsimd.collective_compute(
                                "AllGather", OP.bypass, replica_groups=groups,
                                ins=[cc_kin[l][:]], outs=[cc_kout[l][:]])

                            # V token-major with ones column per head
                            for t in range(NT):
                                ps = psum([128, 512], "E")
                                for fc in range(FC):
                                    nc.tensor.matmul(
                                        out=ps[:],
                                        lhsT=xT[:, fc, t * 128:(t + 1) * 128],
                                        rhs=wv_sb[:, fc, :],
                                        start=(fc == 0), stop=(fc == FC - 1))
                                nc.vector.tensor_add(
                                    out=v_sb[:, t, :, 0:HD],
                                    in0=ps[:].rearrange("p (h d) -> p h d", h=H),
                                    in1=bv_bc[:].rearrange("p (h d) -> p h d", h=H))
                                nc.vector.tensor_copy(
                                    out=v_sb[:, t, :, HD:HD + 1],
                                    in_=onecol[:].broadcast_to([128, H, 1]))
                            nc.sync.dma_start(
                                out=cc_vin[l].rearrange("(t p) c -> p t c", p=128),
                                in_=v_sb[:, 0:NT, :, :])
                            nc.gpsimd.collective_compute(
                                "AllGather", OP.bypass, replica_groups=groups,
                                ins=[cc_vin[l][:]], outs=[cc_vout[l][:]])

                            # Q^T
                            for m in range(FC):
                                for qc in range(2):
                                    ps = psum([128, 512], "E")
                                    for fc in range(FC):
                                        nc.tensor.matmul(
                                            out=ps[:],
                                            lhsT=wq_sb[:, fc, m * 128:(m + 1) * 128],
                                            rhs=xT[:, fc, qc * 512:(qc + 1) * 512],
                                            start=(fc == 0), stop=(fc == FC - 1))
                                    nc.vector.tensor_scalar(
                                        out=q_sb[:, m, qc * 512:(qc + 1) * 512],
                                        in0=ps[:], scalar1=bq_sb[:, m:m + 1],
                                        scalar2=None, op0=OP.add)

                        # bring in the pair's K/V halves (both slabs; own half
                        # rewrite is redundant but keeps the program SPMD).
                        for hh in range(2):
                            nc.sync.dma_start(
                                out=k_sb[:, :, hh * TLOC:(hh + 1) * TLOC],
                                in_=cc_kout[l][hh].rearrange("(c p) t -> p c t", p=128))
                            nc.sync.dma_start(
                                out=v_sb[:, hh * NT:(hh + 1) * NT, :, :],
                                in_=cc_vout[l][hh].rearrange("(t p) c -> p t c", p=128))

                        # ---------------- attention ----------------
                        with tc.tile_pool(name=f"at{l}", bufs=1) as atp:
                            for hp in range(FC):          # head pair = fchunk
                                for hb in range(2):       # head within pair
                                    h = 2 * hp + hb
                                    lo, hi = hb * 64, hb * 64 + 64
                                    ops_ = psum([65, TLOC], "O")
                                    for kt in range(NKT):
                                        eps_ps = psum([128, TLOC], "E")
                                        for qc in range(2):
                                            nc.tensor.matmul(
                                                out=eps_ps[:, qc * 512:(qc + 1) * 512],
                                                lhsT=k_sb[lo:hi, hp, kt * 128:(kt + 1) * 128],
                                                rhs=q_sb[lo:hi, hp, qc * 512:(qc + 1) * 512],
                                                start=True, stop=True,
                                                tile_position=(lo, 0))
                                        p_sb = atp.tile([128, TLOC], F32R, tag="P", bufs=3)
                                        nc.scalar.activation(
                                            out=p_sb[:], in_=eps_ps[:], func=AF.Exp,
                                            bias=mb_t[:, kt, :], scale=SCALE)
                                        for qc in range(2):
                                            nc.tensor.matmul(
                                                out=ops_[:, qc * 512:(qc + 1) * 512],
                                                lhsT=v_sb[:, kt, h, :],
                                                rhs=p_sb[:, qc * 512:(qc + 1) * 512],
                                                start=(kt == 0), stop=(kt == NKT - 1))
                                    # normalize: r = 1/rowsum, broadcast via K=1 mm
                                    r_sb = atp.tile([1, TLOC], F32R, tag="r")
                                    with nc.allow_low_precision(
                                            reason="fp32r rounding of softmax denom"):
                                        nc.vector.reciprocal(out=r_sb[:],
                                                             in_=ops_[64:65, :])
                                    rb_ps = psum([64, TLOC], "E")
                                    for qc in range(2):
                                        nc.tensor.matmul(
                                            out=rb_ps[:, qc * 512:(qc + 1) * 512],
                                            lhsT=ones_r[:],
                                            rhs=r_sb[:, qc * 512:(qc + 1) * 512],
                                            start=True, stop=True)
                                    rb_sb = atp.tile([64, TLOC], F32, tag="rb")
                                    nc.vector.tensor_copy(out=rb_sb[:], in_=rb_ps[:])
                                    nc.vector.tensor_mul(
                                        out=oT[lo:hi, hp, :], in0=ops_[0:64, :],
                                        in1=rb_sb[:])

                    # ---------------- Wo + residual ----------------
                    with tc.tile_pool(name=f"wop{l}", bufs=2) as wop:
                        for t in range(NT):
                            ps = psum([128, 512], "E")
                            for fc in range(FC):
                                nc.tensor.matmul(
                                    out=ps[:],
                                    lhsT=oT[:, fc, t * 128:(t + 1) * 128],
                                    rhs=wo_sb[:, fc, :],
                                    start=(fc == 0), stop=(fc == FC - 1))
                            tmp = wop.tile([128, D], F32, tag="tmp")
                            nc.vector.tensor_add(out=tmp[:], in0=ps[:], in1=bo_bc[:])
                            nc.vector.tensor_add(out=x_t[:, t, :], in0=x_t[:, t, :],
                                                 in1=tmp[:])
                        # LN1 in place
                        for t in range(NT):
                            stats = wop.tile([128, 6], F32, tag="stats")
                            nc.vector.bn_stats(out=stats[:], in_=x_t[:, t, :])
                            mv = wop.tile([128, 2], F32, tag="mv")
                            nc.vector.bn_aggr(out=mv[:], in_=stats[:])
                            rstd = wop.tile([128, 1], F32, tag="rstd")
                            nc.scalar.activation(out=rstd[:], in_=mv[:, 1:2],
                                                 func=AF.Sqrt, bias=eps_t[:])
                            nc.vector.reciprocal(out=rstd[:], in_=rstd[:])
                            nc.vector.tensor_scalar(
                                out=x_t[:, t, :], in0=x_t[:, t, :],
                                scalar1=mv[:, 0:1], scalar2=rstd[:],
                                op0=OP.subtract, op1=OP.mult)
                            nc.vector.tensor_mul(out=x_t[:, t, :], in0=x_t[:, t, :],
                                                 in1=g1_bc[:])
                            nc.vector.tensor_add(out=x_t[:, t, :], in0=x_t[:, t, :],
                                                 in1=be1_bc[:])

                    # ---------------- FFN ----------------
                    with tc.tile_pool(name=f"ff{l}", bufs=1) as ffp:
                        x1T = ffp.tile([128, FC, TLOC], F32R, tag="x1T")
                        for t in range(NT):
                            for fc in range(FC):
                                tp = psum([128, 128], "E")
                                nc.tensor.transpose(
                                    out=tp[:], in_=x_t[:, t, fc * 128:(fc + 1) * 128],
                                    identity=ident[:])
                                nc.vector.tensor_copy(
                                    out=x1T[:, fc, t * 128:(t + 1) * 128], in_=tp[:])
                        h_sb = ffp.tile([128, HC, TLOC], F32R, tag="h")
                        w2_sb = ffp.tile([128, HC, D], F32R, tag="w2")
                        nc.sync.dma_start(
                            out=w2_sb[:],
                            in_=w2_ext[l].rearrange("(c p) m -> p c m", p=128).bitcast(F32R))
                        for ht in range(HC):
                            w1c = ffp.tile([128, FC, 128], F32R, tag="w1c", bufs=3)
                            nc.sync.dma_start(
                                out=w1c[:],
                                in_=w1_ext[l].rearrange(
                                    "(c p) (t m) -> p c t m", p=128, m=128
                                )[:, :, ht, :].bitcast(F32R))
                            hp_ps = psum([128, TLOC], "E")
                            for qc in range(2):
                                for fc in range(FC):
                                    nc.tensor.matmul(
                                        out=hp_ps[:, qc * 512:(qc + 1) * 512],
                                        lhsT=w1c[:, fc, :],
                                        rhs=x1T[:, fc, qc * 512:(qc + 1) * 512],
                                        start=(fc == 0), stop=(fc == FC - 1))
                            # bias + relu fused on DVE
                            nc.vector.tensor_scalar(
                                out=h_sb[:, ht, :], in0=hp_ps[:],
                                scalar1=b1_sb[:, ht:ht + 1], scalar2=0.0,
                                op0=OP.add, op1=OP.max)
                        for t in range(NT):
                            ps = psum([128, 512], "E")
                            for hc in range(HC):
                                nc.tensor.matmul(
                                    out=ps[:],
                                    lhsT=h_sb[:, hc, t * 128:(t + 1) * 128],
                                    rhs=w2_sb[:, hc, :],
                                    start=(hc == 0), stop=(hc == HC - 1))
                            tmp = ffp.tile([128, D], F32, tag="tmp", bufs=2)
                            nc.vector.tensor_add(out=tmp[:], in0=ps[:], in1=b2_bc[:])
                            nc.vector.tensor_add(out=x_t[:, t, :], in0=x_t[:, t, :],
                                                 in1=tmp[:])
                        # LN2 in place
                        for t in range(NT):
                            stats = ffp.tile([128, 6], F32, tag="stats")
                            nc.vector.bn_stats(out=stats[:], in_=x_t[:, t, :])
                            mv = ffp.tile([128, 2], F32, tag="mv")
                            nc.vector.bn_aggr(out=mv[:], in_=stats[:])
                            rstd = ffp.tile([128, 1], F32, tag="rstd")
                            nc.scalar.activation(out=rstd[:], in_=mv[:, 1:2],
                                                 func=AF.Sqrt, bias=eps_t[:])
                            nc.vector.reciprocal(out=rstd[:], in_=rstd[:])
                            nc.vector.tensor_scalar(
                                out=x_t[:, t, :], in0=x_t[:, t, :],
                                scalar1=mv[:, 0:1], scalar2=rstd[:],
                                op0=OP.subtract, op1=OP.mult)
                            nc.vector.tensor_mul(out=x_t[:, t, :], in0=x_t[:, t, :],
                                                 in1=g2_bc[:])
                            nc.vector.tensor_add(out=x_t[:, t, :], in0=x_t[:, t, :],
                                                 in1=be2_bc[:])

            # ---------------- output ----------------
            for t in range(NT):
                nc.sync.dma_start(out=out_ext[t], in_=x_t[:, t, :])

    nc.compile()
    return nc


def _get_program():
    global _PROGRAM
    if _PROGRAM is None:
        _PROGRAM = _build_program()
    return _PROGRAM


def _pos_encoding():
    pos = np.arange(S, dtype=np.float32)[:, None]
    div = np.exp(np.arange(0, D, 2, dtype=np.float32)
                 * np.float32(-np.log(10000.0) / D))
    ang = (pos * div).astype(np.float32)
    out = np.empty((S, D), dtype=np.float32)
    out[:, 0::2] = np.sin(ang)
    out[:, 1::2] = np.cos(ang)
    return out


def _shard_inputs(inputs):
    emb_s = (inputs["tok_emb"].astype(np.float32) * np.float32(np.sqrt(D)))
    emb_s = np.ascontiguousarray(emb_s)
    pos = _pos_encoding()
    ones64 = np.ones((1, 64), dtype=np.float32)
    shared = {"tok_emb": emb_s, "ones64": ones64}
    for nm in ("Wq", "bq", "Wk", "bk", "Wv", "bv", "Wo", "bo",
               "W1", "b1", "W2", "b2", "g1", "be1", "g2", "be2"):
        shared[nm] = np.ascontiguousarray(inputs[nm].astype(np.float32))
    src = np.asarray(inputs["src"])
    mask = np.asarray(inputs["src_mask"])
    in_maps = []
    for c in range(NCORES):
        b, hf = c // 2, c % 2
        m = dict(shared)
        sl = src[b, hf * TLOC:(hf + 1) * TLOC].astype(np.int32)
        m["src_idx"] = np.ascontiguousarray(sl.reshape(NT, 128, 1))
        m["posenc"] = np.ascontiguousarray(
            pos[hf * TLOC:(hf + 1) * TLOC].reshape(NT, 128, D))
        mb = np.where(mask[b, 0, 0, :] == 0, np.float32(-1e10),
                      np.float32(0.0)).astype(np.float32)
        m["maskb"] = np.ascontiguousarray(mb.reshape(NKT, 128, 1))
        in_maps.append(m)
    return in_maps


def _run(inputs, trace=False):
    from concourse.bass_utils import run_bass_kernel_spmd
    nc = _get_program()
    in_maps = _shard_inputs(inputs)
    res = run_bass_kernel_spmd(nc, in_maps, list(range(NCORES)), trace=trace)
    out = np.empty((B, S, D), dtype=np.float32)
    for c in range(NCORES):
        b, hf = c // 2, c % 2
        out[b, hf * TLOC:(hf + 1) * TLOC, :] = \
            res.results[c]["out"].reshape(TLOC, D)
    return out, res


def kernel(**inputs) -> np.ndarray:
    out, _ = _run(inputs, trace=False)
    return out


def make_timed_runner(inputs):
    """Build a repeat-callable with device-resident inputs for timing.

    Mirrors bass2jax.run_bass_via_pjrt's multi-core branch, but keeps the
    big inputs on device across calls so per-call wall ~= dispatch + exec.
    """
    import jax
    import concourse.mybir as mybir
    from concourse import bass2jax
    from concourse.bass2jax import _bass_exec_p, partition_id_tensor
    from jax.sharding import Mesh, PartitionSpec, NamedSharding
    try:
        from jax.experimental.shard_map import shard_map
    except ImportError:
        from jax.sharding import shard_map

    nc = _get_program()
    in_maps = _shard_inputs(inputs)
    partition_name = nc.partition_id_tensor.name if nc.partition_id_tensor else None

    in_names, out_names, out_avals, zero_outs = [], [], [], []
    for alloc in nc.m.functions[0].allocations:
        if not isinstance(alloc, mybir.MemoryLocationSet):
            continue
        name = alloc.memorylocations[0].name
        if alloc.kind == "ExternalInput":
            if name != partition_name:
                in_names.append(name)
        elif alloc.kind == "ExternalOutput":
            out_names.append(name)
            shape = tuple(alloc.tensor_shape)
            dtype = mybir.dt.np(alloc.dtype)
            out_avals.append(jax.core.ShapedArray(shape, dtype))
            zero_outs.append(np.zeros(shape, dtype))
    n_params = len(in_names)
    n_outs = len(out_avals)
    all_in_names = list(in_names) + out_names
    if partition_name is not None:
        all_in_names.append(partition_name)
    donate = tuple(range(n_params, n_params + n_outs))

    def _body(*args):
        operands = list(args)
        if partition_name is not None:
            operands.append(partition_id_tensor())
        outs = _bass_exec_p.bind(
            *operands,
            out_avals=tuple(out_avals),
            in_names=tuple(all_in_names),
            out_names=tuple(out_names),
            lowering_input_output_aliases=(),
            sim_require_finite=True,
            sim_require_nnan=True,
            nc=nc,
        )
        return tuple(outs)

    devices = jax.devices()[:NCORES]
    mesh = Mesh(np.asarray(devices), ("core",))
    in_specs = (PartitionSpec("core"),) * (n_params + n_outs)
    out_specs = (PartitionSpec("core"),) * len(out_names)
    sharded = jax.jit(
        shard_map(_body, mesh=mesh, in_specs=in_specs, out_specs=out_specs,
                  check_rep=False),
        donate_argnums=donate, keep_unused=True)

    shard = NamedSharding(mesh, PartitionSpec("core"))
    dev_in = []
    for i in range(n_params):
        nmi = in_names[i]
        cat = np.concatenate([np.asarray(m[nmi]) for m in in_maps], axis=0)
        dev_in.append(jax.device_put(cat, shard))

    def run_once():
        zeros = [np.zeros((NCORES * z.shape[0], *z.shape[1:]), z.dtype)
                 for z in zero_outs]
        outs = sharded(*dev_in, *zeros)
        for o in outs:
            o.block_until_ready()
        return outs

    return run_once, out_names, out_avals
